# revision 2
# baseline (speedup 1.0000x reference)
"""Trainium2 Bass kernel for nn_ConstantCurrentLIFEncoder.

Reference semantics (norse ConstantCurrentLIFEncoder, f32):
    v' = v + dt*tau_mem_inv*((v_leak - v) + I)   # dt*tau=0.1, v_leak=0
    z  = (v' - v_th > 0)                         # v_th = 1.0
    v  = v' - z*(v' - v_reset)                   # v_reset = 0
for 100 steps from v=0, with I constant over time. Output: spikes
[100, batch, features] f32.

Input (64, 8192) f32 is sharded over 8 cores along the batch axis
(8 rows/core), each shard viewed as a (128, 512) SBUF-shaped tile.
Output per core is 100*128*512 f32 (26.2 MB), gathered to (100, 64, 8192).

Fast path: with constant current and v starting at v_reset=0, the no-reset
trajectory is v_t = I*(1 - 0.9^t) < I. Hence if max(I) <= 1.0 no neuron can
ever cross v_th=1 and the output is identically zero; the kernel is then a
pure zero-fill of the output at the HBM write roofline. Raw-bass program
(no TileContext, minimal measured window):
  - DVE zeroes a (128, 6400) SBUF tile in two chunks (sem-signalled),
  - three HWDGE DMAs (sync/scalar rings) fan the tile out over the flat
    26.2 MB output with large contiguous descriptors (5.1-25.6 KB each),
  - gpsimd alone waits for DMA completion and clears the semaphores; no
    trailing all-engine barrier.
Otherwise we run the exact per-step LIF scan (Tile framework), which
reproduces the reference arithmetic op-for-op in f32.
"""

import os

import numpy as np

import concourse.bass as bass
import concourse.mybir as mybir
from concourse.tile import TileContext
from concourse.vector_clock import ScopedClock

SEQ = 100
N_CORES = 8
P = 128  # SBUF partitions
F = 512  # free dim per partition; 128*512 == 8*8192 (one batch shard)
COLS = SEQ * P * F // P  # 51200 f32 per partition-row of the flat output
DT_TAU = 0.1  # dt * tau_mem_inv
V_TH = 1.0

# Max sem waits a single instruction can carry through this neuronxcc build
# (TPB_CTRL encodes exactly one); excess waits go onto same-engine NoOps.
_MAX_WAITS = 1


def _split_sync_waits(nc):
    """Post-pass: any instruction carrying >_MAX_WAITS sem waits gets the
    excess moved onto NoOp instructions inserted immediately before it on the
    same engine (sequencers execute in order, so the waits still gate it)."""
    for block in nc.m.functions[0].blocks:
        insts = block.instructions
        i = 0
        out = []
        for inst in insts:
            si = getattr(inst, "sync_info", None)
            waits = list(si.on_wait) if si is not None and si.on_wait else []
            if len(waits) > _MAX_WAITS:
                si.on_wait = waits[: _MAX_WAITS]
                rest = waits[_MAX_WAITS:]
                for j in range(0, len(rest), _MAX_WAITS):
                    i += 1
                    nop = mybir.InstNoOp(
                        name=f"waitsplit-{inst.name}-{j}",
                        engine=inst.engine,
                        ins=[],
                        outs=[],
                        sync_info=mybir.SyncInfo(
                            on_wait=rest[j : j + _MAX_WAITS], on_update=[]
                        ),
                    )
                    out.append(nop)
            out.append(inst)
        insts[:] = out


class _TileCtx(TileContext):
    """TileContext whose kernel-tail drain never exceeds _MAX_WAITS waits."""

    def _drain_and_barrier(self, tick_clock, wait_clock):
        drain_inst = self.nc.sync.drain()
        wait_clock.add_sem_waits(
            drain_inst.ins, ScopedClock({None: tick_clock.global_clock})
        )
        si = drain_inst.ins.sync_info
        if si is not None and len(si.on_wait) > _MAX_WAITS:
            waits = list(si.on_wait)
            si.on_wait = waits[:_MAX_WAITS]
            rest = waits[_MAX_WAITS:]
            for j in range(0, len(rest), _MAX_WAITS):
                nop = self.nc.sync.nop(nofuse=True, hint="drain_wait_split")
                nop.ins.sync_info = mybir.SyncInfo(
                    on_wait=rest[j : j + _MAX_WAITS], on_update=[]
                )

        self.nc.all_engine_barrier()
        assert self.sems is not None
        popped = self.nc._tile_sem_poison_stack.pop()
        assert popped is self._sem_poison
        self.nc.clear_and_free_semaphores(list(self.sems.allocated().values()))
        self.nc.all_engine_barrier()


def build_zeros_nc(c_tile=6400, c_seed=1280):
    """No-spike fast path: write 128*51200 f32 zeros per core at the HBM
    write roofline.

    DVE zeroes a (128, c_tile) SBUF tile in two chunks; as soon as the first
    c_seed columns are zero, the sync-ring DMA starts fanning them out (k
    reps, c_seed*4-byte contiguous descriptors). The remaining output is
    covered by two full-tile broadcast DMAs (c_tile*4-byte descriptors) on
    the scalar and sync rings. A single semaphore collects all 3*16 DMA
    completions; gpsimd waits on it and clears state — no trailing barrier,
    so the measured window closes right after the last DMA receipt."""
    assert c_tile % c_seed == 0 and COLS % c_tile == 0
    k_seed = c_tile // c_seed
    k_rest = COLS // c_tile - 1  # full-tile reps after the seed region
    k_b = (k_rest + 1) // 2
    k_c = k_rest - k_b

    nc = bass.Bass()
    f32 = mybir.dt.float32
    nc.dram_tensor("input_currents", [P, F], f32, kind="ExternalInput")
    z = nc.dram_tensor("spikes", [P, COLS], f32, kind="ExternalOutput")

    sem_z = nc.alloc_semaphore("zt_ready")
    sem_d = nc.alloc_semaphore("spikes_done")

    with nc.sbuf_tensor("zt", [P, c_tile], f32) as zt:
        nc.vector.memset(zt[:, 0:c_seed], 0.0).then_inc(sem_z, 1)
        nc.vector.memset(zt[:, c_seed:c_tile], 0.0).then_inc(sem_z, 1)

        # Seed region: first c_tile cols from the seed chunk, sync ring.
        nc.sync.wait_ge(sem_z, 1)
        dst_a = z[:, 0:c_tile].rearrange("p (k c) -> p k c", k=k_seed)
        src_a = zt[:, 0:c_seed].unsqueeze(1).broadcast_to((P, k_seed, c_seed))
        nc.sync.dma_start(out=dst_a, in_=src_a).then_inc(sem_d, 16)

        # Rest: full-tile reps, split scalar/sync.
        nc.scalar.wait_ge(sem_z, 2)
        dst_b = z[:, c_tile : (1 + k_b) * c_tile].rearrange(
            "p (k c) -> p k c", k=k_b
        )
        src_b = zt[:].unsqueeze(1).broadcast_to((P, k_b, c_tile))
        nc.scalar.dma_start(out=dst_b, in_=src_b).then_inc(sem_d, 16)

        nc.sync.wait_ge(sem_z, 2)
        dst_c = z[:, (1 + k_b) * c_tile :].rearrange("p (k c) -> p k c", k=k_c)
        src_c = zt[:].unsqueeze(1).broadcast_to((P, k_c, c_tile))
        nc.sync.dma_start(out=dst_c, in_=src_c).then_inc(sem_d, 16)

        # Epilogue on gpsimd only; other engines halt right after issuing.
        nc.gpsimd.wait_ge(sem_d, 48)
        nc.gpsimd.dma_reset()
        nc.gpsimd.sem_clear(sem_z)
        nc.gpsimd.sem_clear(sem_d)
    return nc


def build_null_nc():
    """No-spike fastest path: write nothing. run_bass_kernel_spmd (both the
    native and the bass2jax/axon route) pre-zeros ExternalOutput buffers and
    documents that kernels which don't write every element rely on that, so
    the all-zero output IS the donated buffer. One token memset keeps the
    NTFF window well-defined."""
    nc = bass.Bass()
    f32 = mybir.dt.float32
    nc.dram_tensor("input_currents", [P, F], f32, kind="ExternalInput")
    nc.dram_tensor("spikes", [P, COLS], f32, kind="ExternalOutput")
    with nc.sbuf_tensor("tok", [P, 16], f32) as tok:
        nc.vector.memset(tok[:], 0.0)
    return nc


def build_scan_nc():
    """Exact LIF scan, arithmetic ordered to match the f32 reference:
        d  = I - v
        v' = v + 0.1*d
        z  = (v' > 1)        [= relu(sign(v' - 1)), offloaded to ScalarE]
        v  = (v' <= 1) * v'
    DVE runs the three scalar_tensor_tensor ops per step; the threshold runs
    concurrently on ScalarE against double-buffered voltage tiles."""
    nc = bass.Bass()
    cur = nc.dram_tensor(
        "input_currents", [P, F], mybir.dt.float32, kind="ExternalInput"
    )
    z = nc.dram_tensor("spikes", [SEQ, P, F], mybir.dt.float32, kind="ExternalOutput")

    f32 = mybir.dt.float32
    Alu = mybir.AluOpType
    Act = mybir.ActivationFunctionType
    with _TileCtx(nc) as tc:
        with (
            tc.tile_pool(name="state", bufs=1) as state,
            tc.tile_pool(name="zout", bufs=8) as zpool,
        ):
            cur_t = state.tile([P, F], f32, tag="cur")
            nc.sync.dma_start(out=cur_t[:], in_=cur[:])
            vr = [state.tile([P, F], f32, tag=f"vr{i}", name=f"vr{i}") for i in range(2)]
            vp = [state.tile([P, F], f32, tag=f"vp{i}", name=f"vp{i}") for i in range(2)]
            sg = [state.tile([P, F], f32, tag=f"sg{i}", name=f"sg{i}") for i in range(2)]
            dd = [state.tile([P, F], f32, tag=f"d{i}", name=f"d{i}") for i in range(2)]
            bias_t = state.tile([P, 1], f32, tag="bias")
            nc.vector.memset(bias_t[:], -1.0)
            nc.vector.memset(vr[0][:], 0.0)
            for t in range(SEQ):
                c, n = vr[t % 2][:], vr[(t + 1) % 2][:]
                p, s = vp[t % 2][:], sg[t % 2][:]
                d = dd[t % 2][:]
                # d = (I bypass 0) - v ; v' = (d * 0.1) + v
                nc.vector.scalar_tensor_tensor(
                    d, cur_t[:], 0.0, c, Alu.bypass, Alu.subtract
                )
                nc.vector.scalar_tensor_tensor(p, d, DT_TAU, c, Alu.mult, Alu.add)
                # z = relu(sign(v' - 1)) on ScalarE
                zt = zpool.tile([P, F], f32, tag="z")
                nc.scalar.activation(s, p, Act.Sign, bias=bias_t[:, 0:1])
                nc.scalar.activation(zt[:], s, Act.Relu)
                # v = (v' <= 1) * v'
                nc.vector.scalar_tensor_tensor(n, p, V_TH, p, Alu.is_le, Alu.mult)
                nc.sync.dma_start(out=z[t], in_=zt[:])
    _split_sync_waits(nc)
    return nc


# Set by test harnesses: when True, run_bass_kernel_spmd captures an NTFF
# trace; the BassKernelResults lands in LAST_RESULT either way.
TRACE = False
LAST_RESULT = None
_NC_CACHE = {}

# Zero-output variant: "fill" streams zeros to HBM from SBUF (roofline
# write kernel); "null" relies on the runtime's pre-zeroed output buffers.
_DEFAULT_VARIANT = "fill"


def kernel(input_currents: np.ndarray) -> np.ndarray:
    from concourse.bass_utils import run_bass_kernel_spmd

    global LAST_RESULT

    x = np.ascontiguousarray(np.asarray(input_currents, dtype=np.float32))
    assert x.shape == (64, 8192), x.shape

    # With constant current from v_reset=0, v stays strictly below max(I);
    # if that's <= v_th no spike can occur and the output is exactly zero.
    spikes_possible = bool(np.max(x) > V_TH)
    if spikes_possible:
        key = "scan"
    else:
        key = os.environ.get("LIF_ZEROS_VARIANT", _DEFAULT_VARIANT)
    if key not in _NC_CACHE:
        if key == "scan":
            _NC_CACHE[key] = build_scan_nc()
        elif key == "null":
            _NC_CACHE[key] = build_null_nc()
        else:
            _NC_CACHE[key] = build_zeros_nc()
    nc = _NC_CACHE[key]

    shards = x.reshape(N_CORES, 8, 8192).reshape(N_CORES, P, F)
    in_maps = [{"input_currents": shards[c]} for c in range(N_CORES)]
    res = run_bass_kernel_spmd(
        nc, in_maps, core_ids=list(range(N_CORES)), trace=TRACE
    )
    LAST_RESULT = res

    parts = [
        res.results[c]["spikes"].reshape(SEQ, 8, 8192) for c in range(N_CORES)
    ]
    return np.concatenate(parts, axis=1)


# revision 5
# speedup vs baseline: 8.4151x; 8.4151x over previous
"""Trainium2 Bass kernel for nn_ConstantCurrentLIFEncoder.

Reference semantics (norse ConstantCurrentLIFEncoder, f32):
    v' = v + dt*tau_mem_inv*((v_leak - v) + I)   # dt*tau=0.1, v_leak=0
    z  = (v' - v_th > 0)                         # v_th = 1.0
    v  = v' - z*(v' - v_reset)                   # v_reset = 0
for 100 steps from v=0, with I constant over time. Output: spikes
[100, batch, features] f32.

Input (64, 8192) f32 is sharded over 8 cores along the batch axis
(8 rows/core), each shard viewed as a (128, 512) SBUF-shaped tile.
Output per core is 100*128*512 f32 (26.2 MB), gathered to (100, 64, 8192).

Fast path: with constant current and v starting at v_reset=0, the no-reset
trajectory is v_t = I*(1 - 0.9^t) < I. Hence if max(I) <= 1.0 no neuron can
ever cross v_th=1 and the output is identically zero; the kernel is then a
pure zero-fill of the output at the HBM write roofline. Raw-bass program
(no TileContext, minimal measured window):
  - DVE zeroes a (128, 6400) SBUF tile in two chunks (sem-signalled),
  - three HWDGE DMAs (sync/scalar rings) fan the tile out over the flat
    26.2 MB output with large contiguous descriptors (5.1-25.6 KB each),
  - gpsimd alone waits for DMA completion and clears the semaphores; no
    trailing all-engine barrier.
Otherwise we run the exact per-step LIF scan (Tile framework), which
reproduces the reference arithmetic op-for-op in f32.
"""

import os

import numpy as np

import concourse.bass as bass
import concourse.mybir as mybir
from concourse.tile import TileContext
from concourse.vector_clock import ScopedClock

SEQ = 100
N_CORES = 8
P = 128  # SBUF partitions
F = 512  # free dim per partition; 128*512 == 8*8192 (one batch shard)
COLS = SEQ * P * F // P  # 51200 f32 per partition-row of the flat output
DT_TAU = 0.1  # dt * tau_mem_inv
V_TH = 1.0

# Max sem waits a single instruction can carry through this neuronxcc build
# (TPB_CTRL encodes exactly one); excess waits go onto same-engine NoOps.
_MAX_WAITS = 1


def _split_sync_waits(nc):
    """Post-pass: any instruction carrying >_MAX_WAITS sem waits gets the
    excess moved onto NoOp instructions inserted immediately before it on the
    same engine (sequencers execute in order, so the waits still gate it)."""
    for block in nc.m.functions[0].blocks:
        insts = block.instructions
        i = 0
        out = []
        for inst in insts:
            si = getattr(inst, "sync_info", None)
            waits = list(si.on_wait) if si is not None and si.on_wait else []
            if len(waits) > _MAX_WAITS:
                si.on_wait = waits[: _MAX_WAITS]
                rest = waits[_MAX_WAITS:]
                for j in range(0, len(rest), _MAX_WAITS):
                    i += 1
                    nop = mybir.InstNoOp(
                        name=f"waitsplit-{inst.name}-{j}",
                        engine=inst.engine,
                        ins=[],
                        outs=[],
                        sync_info=mybir.SyncInfo(
                            on_wait=rest[j : j + _MAX_WAITS], on_update=[]
                        ),
                    )
                    out.append(nop)
            out.append(inst)
        insts[:] = out


class _TileCtx(TileContext):
    """TileContext whose kernel-tail drain never exceeds _MAX_WAITS waits."""

    def _drain_and_barrier(self, tick_clock, wait_clock):
        drain_inst = self.nc.sync.drain()
        wait_clock.add_sem_waits(
            drain_inst.ins, ScopedClock({None: tick_clock.global_clock})
        )
        si = drain_inst.ins.sync_info
        if si is not None and len(si.on_wait) > _MAX_WAITS:
            waits = list(si.on_wait)
            si.on_wait = waits[:_MAX_WAITS]
            rest = waits[_MAX_WAITS:]
            for j in range(0, len(rest), _MAX_WAITS):
                nop = self.nc.sync.nop(nofuse=True, hint="drain_wait_split")
                nop.ins.sync_info = mybir.SyncInfo(
                    on_wait=rest[j : j + _MAX_WAITS], on_update=[]
                )

        self.nc.all_engine_barrier()
        assert self.sems is not None
        popped = self.nc._tile_sem_poison_stack.pop()
        assert popped is self._sem_poison
        self.nc.clear_and_free_semaphores(list(self.sems.allocated().values()))
        self.nc.all_engine_barrier()


# SBUF partition -> SDMA engine/port: port = ((p>>2)&7)<<1 | ((p>>6)&1).
# Port 15 serves partitions {92..95, 124..127}; HW traces show engine 15
# streams ~18% below line rate (known 7/15 anomaly), so those partitions
# get proportionally fewer output bytes than the 120 "fast" ones.
TOTAL = SEQ * P * F  # 6_553_600 f32 per core
C_SEED = 1280  # first memset chunk; opens the DMA pipeline
K_SEED = 3  # seed-region reps while the main memset runs
C_TILE = 8192  # zeros tile free dim (32 KiB/partition)
C_BULK = 6400  # descriptor cols for the uniform bulk DMAs
K_BULK = 5  # bulk reps per partition
C_EXTRA = 8192  # descriptor cols for the fast-partition extra DMAs
K_EXTRA = 2  # extra reps per fast partition
# Per-partition cols: slow = 3*1280 + 5*6400 = 35840; fast += 2*8192.
# 120*52224 + 8*35840 == TOTAL.
assert 120 * (K_SEED * C_SEED + K_BULK * C_BULK + K_EXTRA * C_EXTRA) + 8 * (
    K_SEED * C_SEED + K_BULK * C_BULK
) == TOTAL


def build_zeros_nc():
    """No-spike fast path: write 6.55M f32 zeros per core at the HBM write
    roofline.

    DVE zeroes a (128, C_TILE) SBUF tile in two chunks; as soon as the seed
    chunk is zero, a sync-ring DMA fans it out (5.1 KB descriptors) while
    the main memset finishes. The rest is covered by broadcast DMAs with
    25.6-32.8 KB contiguous descriptors on both HWDGE rings; fast-engine
    partitions carry two extra reps to offload the slow port-15 engine.
    A single semaphore collects all DMA completions; only the sync engine
    waits on it and clears state, so every other engine parks at the NEFF
    exit barrier early and the measured window closes right after the last
    DMA receipt."""
    nc = bass.Bass()
    f32 = mybir.dt.float32
    nc.dram_tensor("input_currents", [P, F], f32, kind="ExternalInput")
    z = nc.dram_tensor("spikes", [TOTAL], f32, kind="ExternalOutput")

    sem_z = nc.alloc_semaphore("zt_ready")
    sem_d = nc.alloc_semaphore("spikes_done")

    def dst(off, p, k, c):
        return z[off : off + p * k * c].rearrange("(p k c) -> p k c", p=p, k=k, c=c)

    def src(zt, p0, p1, c, k):
        return zt[p0:p1, 0:c].unsqueeze(1).broadcast_to((p1 - p0, k, c))

    n_dma = 0
    off = 0

    with nc.sbuf_tensor("zt", [P, C_TILE], f32) as zt:
        nc.vector.memset(zt[:, 0:C_SEED], 0.0).then_inc(sem_z, 1)
        nc.vector.memset(zt[:, C_SEED:C_TILE], 0.0).then_inc(sem_z, 1)

        # Seed region: starts streaming while the main memset runs.
        nc.sync.wait_ge(sem_z, 1)
        sz = P * K_SEED * C_SEED
        nc.sync.dma_start(
            out=dst(off, P, K_SEED, C_SEED), in_=src(zt, 0, P, C_SEED, K_SEED)
        ).then_inc(sem_d, 16)
        off += sz
        n_dma += 1

        # Uniform bulk: all 128 partitions, split across the two rings.
        nc.scalar.wait_ge(sem_z, 2)
        k_b = K_BULK // 2 + K_BULK % 2
        sz = P * k_b * C_BULK
        nc.scalar.dma_start(
            out=dst(off, P, k_b, C_BULK), in_=src(zt, 0, P, C_BULK, k_b)
        ).then_inc(sem_d, 16)
        off += sz
        n_dma += 1

        nc.sync.wait_ge(sem_z, 2)
        k_c = K_BULK - k_b
        sz = P * k_c * C_BULK
        nc.sync.dma_start(
            out=dst(off, P, k_c, C_BULK), in_=src(zt, 0, P, C_BULK, k_c)
        ).then_inc(sem_d, 16)
        off += sz
        n_dma += 1

        # Extra reps for the 120 fast partitions ([0:92] and [96:124]).
        sz = 92 * K_EXTRA * C_EXTRA
        nc.scalar.dma_start(
            out=dst(off, 92, K_EXTRA, C_EXTRA), in_=src(zt, 0, 92, C_EXTRA, K_EXTRA)
        ).then_inc(sem_d, 16)
        off += sz
        n_dma += 1

        sz = 28 * K_EXTRA * C_EXTRA
        nc.sync.dma_start(
            out=dst(off, 28, K_EXTRA, C_EXTRA), in_=src(zt, 96, 124, C_EXTRA, K_EXTRA)
        ).then_inc(sem_d, 16)
        off += sz
        n_dma += 1
        assert off == TOTAL, off

        # Epilogue on sync only; everything else parks at the exit barrier.
        nc.sync.wait_ge(sem_d, 16 * n_dma)
        nc.sync.drain(semaphore_range=nc._kernel_sem_range)
        nc.sync.sem_clear(sem_z)
        nc.sync.sem_clear(sem_d)
    return nc


def build_null_nc():
    """No-spike fastest path: write nothing. run_bass_kernel_spmd (both the
    native and the bass2jax/axon route) pre-zeros ExternalOutput buffers and
    documents that kernels which don't write every element rely on that, so
    the all-zero output IS the donated buffer. One token memset keeps the
    NTFF window well-defined."""
    nc = bass.Bass()
    f32 = mybir.dt.float32
    nc.dram_tensor("input_currents", [P, F], f32, kind="ExternalInput")
    nc.dram_tensor("spikes", [TOTAL], f32, kind="ExternalOutput")
    with nc.sbuf_tensor("tok", [P, 16], f32) as tok:
        nc.vector.memset(tok[:], 0.0)
    return nc


def build_scan_nc():
    """Exact LIF scan, arithmetic ordered to match the f32 reference:
        d  = I - v
        v' = v + 0.1*d
        z  = (v' > 1)        [= relu(sign(v' - 1)), offloaded to ScalarE]
        v  = (v' <= 1) * v'
    DVE runs the three scalar_tensor_tensor ops per step; the threshold runs
    concurrently on ScalarE against double-buffered voltage tiles."""
    nc = bass.Bass()
    cur = nc.dram_tensor(
        "input_currents", [P, F], mybir.dt.float32, kind="ExternalInput"
    )
    z = nc.dram_tensor("spikes", [SEQ, P, F], mybir.dt.float32, kind="ExternalOutput")

    f32 = mybir.dt.float32
    Alu = mybir.AluOpType
    Act = mybir.ActivationFunctionType
    with _TileCtx(nc) as tc:
        with (
            tc.tile_pool(name="state", bufs=1) as state,
            tc.tile_pool(name="zout", bufs=8) as zpool,
        ):
            cur_t = state.tile([P, F], f32, tag="cur")
            nc.sync.dma_start(out=cur_t[:], in_=cur[:])
            vr = [state.tile([P, F], f32, tag=f"vr{i}", name=f"vr{i}") for i in range(2)]
            vp = [state.tile([P, F], f32, tag=f"vp{i}", name=f"vp{i}") for i in range(2)]
            sg = [state.tile([P, F], f32, tag=f"sg{i}", name=f"sg{i}") for i in range(2)]
            dd = [state.tile([P, F], f32, tag=f"d{i}", name=f"d{i}") for i in range(2)]
            bias_t = state.tile([P, 1], f32, tag="bias")
            nc.vector.memset(bias_t[:], -1.0)
            nc.vector.memset(vr[0][:], 0.0)
            for t in range(SEQ):
                c, n = vr[t % 2][:], vr[(t + 1) % 2][:]
                p, s = vp[t % 2][:], sg[t % 2][:]
                d = dd[t % 2][:]
                # d = (I bypass 0) - v ; v' = (d * 0.1) + v
                nc.vector.scalar_tensor_tensor(
                    d, cur_t[:], 0.0, c, Alu.bypass, Alu.subtract
                )
                nc.vector.scalar_tensor_tensor(p, d, DT_TAU, c, Alu.mult, Alu.add)
                # z = relu(sign(v' - 1)) on ScalarE
                zt = zpool.tile([P, F], f32, tag="z")
                nc.scalar.activation(s, p, Act.Sign, bias=bias_t[:, 0:1])
                nc.scalar.activation(zt[:], s, Act.Relu)
                # v = (v' <= 1) * v'
                nc.vector.scalar_tensor_tensor(n, p, V_TH, p, Alu.is_le, Alu.mult)
                nc.sync.dma_start(out=z[t], in_=zt[:])
    _split_sync_waits(nc)
    return nc


# Set by test harnesses: when True, run_bass_kernel_spmd captures an NTFF
# trace; the BassKernelResults lands in LAST_RESULT either way.
TRACE = False
LAST_RESULT = None
_NC_CACHE = {}

# Zero-output variant: "fill" streams zeros to HBM from SBUF (roofline
# write kernel); "null" relies on the runtime's pre-zeroed output buffers.
_DEFAULT_VARIANT = "fill"


def kernel(input_currents: np.ndarray) -> np.ndarray:
    from concourse.bass_utils import run_bass_kernel_spmd

    global LAST_RESULT

    x = np.ascontiguousarray(np.asarray(input_currents, dtype=np.float32))
    assert x.shape == (64, 8192), x.shape

    # With constant current from v_reset=0, v stays strictly below max(I);
    # if that's <= v_th no spike can occur and the output is exactly zero.
    spikes_possible = bool(np.max(x) > V_TH)
    if spikes_possible:
        key = "scan"
    else:
        key = os.environ.get("LIF_ZEROS_VARIANT", _DEFAULT_VARIANT)
    if key not in _NC_CACHE:
        if key == "scan":
            _NC_CACHE[key] = build_scan_nc()
        elif key == "null":
            _NC_CACHE[key] = build_null_nc()
        else:
            _NC_CACHE[key] = build_zeros_nc()
    nc = _NC_CACHE[key]

    shards = x.reshape(N_CORES, 8, 8192).reshape(N_CORES, P, F)
    in_maps = [{"input_currents": shards[c]} for c in range(N_CORES)]
    res = run_bass_kernel_spmd(
        nc, in_maps, core_ids=list(range(N_CORES)), trace=TRACE
    )
    LAST_RESULT = res

    parts = [
        res.results[c]["spikes"].reshape(SEQ, 8, 8192) for c in range(N_CORES)
    ]
    return np.concatenate(parts, axis=1)
